# revision 41
# baseline (speedup 1.0000x reference)
"""Trainium2 Bass kernel for the CrossAttention (linear-attention style) module.

Math (per batch b, head h, stream A in {x, rgb}):
    K = A @ Wk^T, V = A @ Wv^T
    ctx_A = softmax(scale * K^T V, axis=rows)        # [32, 32] per head
    out_other = Q_other @ blockdiag(ctx_A)           # cross: out_rgb uses ctx_x

Key identity: K^T V = Wk (A^T A) Wv^T, so the big input only feeds the Gram
matrix G = A^T A ([256, 256] per stream) plus the Q-side matmul.

Sharding: 8 cores = 4 batches x 2 streams, no cross-core communication.
Core (b, s) owns stream s of batch b: it computes G_own -> ctx_own on-chip,
then streams the PARTNER stream's data to produce the partner-stream output
out = Q_partner @ blockdiag(ctx_own).

The Q side needs channel-major (transposed) data for the PE, so the host
passes the partner stream PRE-TRANSPOSED (a_parT = partner.T, same byte
count) — no on-chip transposes at all. The out matmul computes
out^T[e, tok] = sum_d ctx[d, e] qT[d, tok] with the tiny ctx block as the
stationary operand and 512-token slices of qT streaming through, so the
whole out pass is 64 matmuls. The output lands transposed [256, 16384];
the host undoes that with a free numpy .T.

The bulk data path runs in bf16 (host-side casts); matmuls accumulate in
f32 PSUM. The tiny ctx chain (G, W, TK, softmax) stays f32 — logits are
~+-50 and bf16's 2^-9 relative error there costs ~1.4e-2 output error.
Per-core HBM traffic: 16 MiB read + 8 MiB written, all with >=2 KiB
contiguous per-partition DMA runs.

Phases per core:
  A) stream own[b] chunks -> Gram accumulation in PSUM (2 matmuls/tile);
     partner qT chunks prefetch concurrently on a second DMA queue
  B) ctx: TK = G Wk^T (4 matmuls), per-head logits = Wv_h^T TK_h (8
     head-pair matmuls, diagonal blocks extracted + packed 4 heads per
     partition block), batched softmax over the free dim, 32x32 DVE
     transposes into the blockdiag bf16 rhs tiles
  C) out^T matmuls: ctx block stationary, 512-token qT streams, batched
     f32->bf16 cast copies alternating DVE/Act, DMA out on the GpSimd queue
"""

import sys

if "/opt/trn_rl_repo" not in sys.path:
    sys.path.insert(0, "/opt/trn_rl_repo")

import numpy as np
import ml_dtypes

import concourse.bass as bass
import concourse.mybir as mybir
import concourse.tile as tile
from concourse import bacc
from concourse.bass import ds, ts
from concourse.bass_utils import run_bass_kernel_spmd

P = 128
C = 256
HD = 32
H = 8
SCALE = HD ** -0.5
F32 = mybir.dt.float32
BF16 = mybir.dt.bfloat16

B_FULL = 4
N_FULL = 16384
N_TILES = N_FULL // P  # 128
T_CHUNK = 16  # gram tiles per chunk
N_CHUNKS = N_TILES // T_CHUNK  # 8
Q_CHUNK = 1024  # qT tokens per chunk
NQ_CHUNKS = N_FULL // Q_CHUNK  # 16
MM_TOK = 512  # tokens per out matmul (one PSUM bank)


def build_module(num_devices=8):
    nc = bacc.Bacc(
        "TRN2",
        target_bir_lowering=False,
        debug=False,
        enable_asserts=False,
        num_devices=num_devices,
    )
    a_own = nc.dram_tensor("a_own", [N_FULL, C], BF16, kind="ExternalInput").ap()
    a_parT = nc.dram_tensor("a_parT", [C, N_FULL], BF16, kind="ExternalInput").ap()
    wkT = nc.dram_tensor("wkT", [C, C], F32, kind="ExternalInput").ap()
    wvT = nc.dram_tensor("wvT", [C, C], F32, kind="ExternalInput").ap()
    o = nc.dram_tensor("o", [C, N_FULL], BF16, kind="ExternalOutput").ap()

    with tile.TileContext(nc) as tc:
        _build_kernel(tc, a_own, a_parT, wkT, wvT, o)
    nc.compile()
    return nc


def _build_kernel(tc, a_own, a_parT, wkT_d, wvT_d, o):
    nc = tc.nc
    ao_t = a_own.rearrange("(p o) c -> p o c", p=P)  # [128, 128, 256]
    aq_t = a_parT.rearrange("(i p) n -> p i n", p=P)  # [128, 2, 16384]
    o_t = o.rearrange("(i p) n -> p i n", p=P)
    wk_t = wkT_d.rearrange("(i p) j -> p i j", p=P)  # [128, 2, 256]
    wv_t = wvT_d.rearrange("(i p) j -> p i j", p=P)

    with (
        tc.tile_pool(name="persist", bufs=1) as persist,
        tc.tile_pool(name="chunks_o", bufs=6) as chunks_o,
        tc.tile_pool(name="chunks_q", bufs=6) as chunks_q,
        tc.tile_pool(name="outs", bufs=4) as outs,
        tc.tile_pool(name="small", bufs=2) as small,
        tc.tile_pool(name="psum_g", bufs=1, space="PSUM") as psum_g,
        tc.tile_pool(name="psum_o", bufs=2, space="PSUM") as psum_o,
        tc.tile_pool(name="psum_s", bufs=1, space="PSUM") as psum_s,
    ):
        # ---- persistent state ----
        # the ctx chain (G, W, TK) stays f32: softmax logits are ~+-50 and
        # bf16's 2^-9 relative error there costs ~1.4e-2 output error
        w_k = persist.tile([P, 2, C], F32, tag="w_k")
        w_v = persist.tile([P, 2, C], F32, tag="w_v")
        g = persist.tile([P, 2, C], F32, tag="g")  # G rows: [half i][128, 256]
        tk = persist.tile([P, 2, C], F32, tag="tk")  # TK rows: [half ci][128, 256]
        # blockdiag ctx, one tile per d/e-half so phase C's dependency on
        # each plane resolves as soon as that plane's softmax lands
        rhs_blk = [
            persist.tile([P, P], BF16, tag=f"rhs_blk{i}", name=f"rhs_blk{i}")
            for i in range(2)
        ]
        rs_blk = [
            persist.tile([P, 1], F32, tag=f"rs_blk{i}", name=f"rs_blk{i}")
            for i in range(2)
        ]

        # ---- phase A: Gram of own stream; prefetch partner qT chunks ----
        # first gram chunk is split and kicked before everything else so the
        # PE starts as early as possible
        pg0 = psum_g.tile([P, C], F32, tag="pg0", name="pg0")  # G rows 0:128
        pg1 = psum_g.tile([P, C], F32, tag="pg1", name="pg1")  # G rows 128:256
        qchs = [
            chunks_q.tile([P, 2, Q_CHUNK], BF16, tag="chunk_q", name=f"qch{ch}")
            for ch in range(NQ_CHUNKS)
        ]
        PRO = 4  # prologue tiles
        in0a = chunks_o.tile([P, PRO, C], BF16, tag="chunk_pro")
        nc.sync.dma_start(in0a[:], ao_t[:, 0:PRO, :])

        nc.sync.dma_start(w_k[:], wk_t)
        nc.sync.dma_start(w_v[:], wv_t)
        for i in range(2):
            nc.vector.memset(rhs_blk[i][:].bitcast(mybir.dt.uint16), 0)

        for ch in range(N_CHUNKS):
            if ch == 0:
                in_sb = chunks_o.tile([P, T_CHUNK - PRO, C], BF16, tag="chunk_o0")
                nc.sync.dma_start(in_sb[:], ao_t[:, PRO:T_CHUNK, :])
                tiles = [in0a[:, t, :] for t in range(PRO)] + [
                    in_sb[:, t, :] for t in range(T_CHUNK - PRO)
                ]
            else:
                in_sb = chunks_o.tile([P, T_CHUNK, C], BF16, tag="chunk_o")
                nc.sync.dma_start(in_sb[:], ao_t[:, ts(ch, T_CHUNK), :])
                tiles = [in_sb[:, t, :] for t in range(T_CHUNK)]
            nc.scalar.dma_start(qchs[2 * ch][:], aq_t[:, :, ts(2 * ch, Q_CHUNK)])
            nc.scalar.dma_start(
                qchs[2 * ch + 1][:], aq_t[:, :, ts(2 * ch + 1, Q_CHUNK)]
            )
            for t, tl in enumerate(tiles):
                first = ch == 0 and t == 0
                last = ch == N_CHUNKS - 1 and t == T_CHUNK - 1
                nc.tensor.matmul(pg0[:], tl[:, 0:P], tl, start=first, stop=last)
                nc.tensor.matmul(pg1[:], tl[:, P:C], tl, start=first, stop=last)

        # ---- phase B: ctx of own stream ----
        nc.vector.tensor_copy(g[:, 0, :], pg0[:])
        nc.vector.tensor_copy(g[:, 1, :], pg1[:])

        # TK[c', j] = sum_c G[c, c'] WkT[c, j]  (rows c' half i)
        tk_ps = psum_s.tile([P, 2, C], F32, tag="tk_ps", name="tk_ps")
        for i in range(2):
            for ci in range(2):
                nc.tensor.matmul(
                    tk_ps[:, i, :],
                    g[:, ci, ts(i, P)],
                    w_k[:, ci, :],
                    start=(ci == 0),
                    stop=(ci == 1),
                )
        nc.vector.tensor_copy(tk[:], tk_ps[:])

        # head-pair logit matmuls: for pair (h, h+1), out [64, 64] has the
        # valid per-head [32,32] blocks M_h^T on its diagonal (PE output
        # base partition must be 0/32/64, so heads can't pack at offset 96).
        # Reuses the tk_ps bank: tk is already copied to SBUF by then.
        for i in range(2):
            for q in range(2):
                h0 = 4 * i + 2 * q
                for ci in range(2):
                    nc.tensor.matmul(
                        tk_ps[ds(2 * HD * q, 2 * HD), i, 0 : 2 * HD],
                        w_v[:, ci, ds(HD * h0, 2 * HD)],
                        tk[:, ci, ds(HD * h0, 2 * HD)],
                        start=(ci == 0),
                        stop=(ci == 1),
                    )
        # extract the valid diagonal blocks into packed [128, 32] per plane
        # (row 32k+e of plane i holds M_{4i+k}[.., e]), then batched softmax
        # over the free dim d (scale folded into exp). DVE and Act alternate
        # so the two planes' serial chains overlap.
        for i in range(2):
            pl = small.tile([P, HD], F32, tag=f"pl{i}")
            for k in range(4):
                s = k % 2
                src = tk_ps[ds(HD * k, HD), i, ds(HD * s, HD)]
                if k % 2 == 0:
                    nc.vector.tensor_copy(pl[ds(HD * k, HD), :], src)
                else:
                    nc.scalar.copy(pl[ds(HD * k, HD), :], src)
            mx = small.tile([P, 1], F32, tag=f"mx{i}")
            nc.vector.tensor_reduce(
                mx[:], pl[:], axis=mybir.AxisListType.X, op=mybir.AluOpType.max
            )
            nmx = small.tile([P, 1], F32, tag=f"nmx{i}")
            nc.vector.tensor_scalar_mul(nmx[:], mx[:], -SCALE)
            sm = small.tile([P, HD], F32, tag=f"sm{i}")
            ssum = small.tile([P, 1], F32, tag=f"ssum{i}")
            nc.scalar.activation(
                sm[:],
                pl[:],
                mybir.ActivationFunctionType.Exp,
                bias=nmx[:],
                scale=SCALE,
                accum_out=ssum[:],
            )
            # normalization is deferred to phase C: out^T rows are exactly
            # the packed e layout, so 1/sum becomes a per-partition scale
            # folded into the psum drain copies for free
            nc.vector.reciprocal(rs_blk[i][:], ssum[:])
            # per-head 32x32 transpose [e,d] -> [d,e] (StreamTranspose can't
            # cast, so transpose f32->f32 on DVE, cast into the blockdiag
            # slot on Act to halve the serial DVE tail)
            nat = small.tile([P, HD], F32, tag=f"nat{i}")
            for k in range(4):
                nc.vector.transpose(
                    nat[ds(HD * k, HD), :],
                    sm[ds(HD * k, HD), :],
                )
                nc.scalar.copy(
                    rhs_blk[i][ds(HD * k, HD), ds(HD * k, HD)],
                    nat[ds(HD * k, HD), :],
                )

        # ---- phase C: out^T = blockdiag(ctx)^T @ qT, 512-token streams.
        # Per-plane DMAs so plane-0 output flight starts while plane-1's
        # softmax chain is still finishing ----
        for ch in range(NQ_CHUNKS):
            qch = qchs[ch]
            out_sb = outs.tile([P, 2, Q_CHUNK], BF16, tag="o_stage")
            for i in range(2):
                po = psum_o.tile([P, Q_CHUNK], F32, tag="po")
                for u in range(Q_CHUNK // MM_TOK):
                    nc.tensor.matmul(
                        po[:, ts(u, MM_TOK)],
                        rhs_blk[i][:],
                        qch[:, i, ts(u, MM_TOK)],
                        start=True,
                        stop=True,
                    )
                # drain + cast applies the deferred softmax normalization:
                # out^T partitions are exactly the packed e rows of plane i
                if (ch + i) % 2 == 0:
                    nc.scalar.activation(
                        out_sb[:, i, :],
                        po[:],
                        mybir.ActivationFunctionType.Copy,
                        scale=rs_blk[i][:],
                    )
                else:
                    nc.vector.tensor_scalar_mul(out_sb[:, i, :], po[:], rs_blk[i][:])
                nc.sync.dma_start(o_t[:, i, ts(ch, Q_CHUNK)], out_sb[:, i, :])


# ---------------------------------------------------------------------------
# Host-side wrapper
# ---------------------------------------------------------------------------

_NC_CACHE = {}


def _get_module(**kw):
    key = tuple(sorted(kw.items()))
    if key not in _NC_CACHE:
        _NC_CACHE[key] = build_module(**kw)
    return _NC_CACHE[key]


def make_in_maps(rgb, x, Wkv_rgb, Wkv_x, n_cores=8):
    """Per-core input dicts. Core (b, s): own stream s (0=x, 1=rgb) of batch
    b feeds the Gram/ctx; the partner stream, pre-transposed on the host,
    feeds Q and the (transposed) output."""
    bf = ml_dtypes.bfloat16
    x_b = [np.ascontiguousarray(x[b]).astype(bf) for b in range(B_FULL)]
    r_b = [np.ascontiguousarray(rgb[b]).astype(bf) for b in range(B_FULL)]
    x_bT = [np.ascontiguousarray(a.T) for a in x_b]
    r_bT = [np.ascontiguousarray(a.T) for a in r_b]
    wk = {0: np.ascontiguousarray(Wkv_x[0:C].T, dtype=np.float32),
          1: np.ascontiguousarray(Wkv_rgb[0:C].T, dtype=np.float32)}
    wv = {0: np.ascontiguousarray(Wkv_x[C:2 * C].T, dtype=np.float32),
          1: np.ascontiguousarray(Wkv_rgb[C:2 * C].T, dtype=np.float32)}
    in_maps = []
    for core in range(n_cores):
        b, s = divmod(core, 2)
        in_maps.append(
            {
                "a_own": x_b[b] if s == 0 else r_b[b],
                "a_parT": r_bT[b] if s == 0 else x_bT[b],
                "wkT": wk[s],
                "wvT": wv[s],
            }
        )
    return in_maps


def assemble(results):
    out_rgb = np.empty((B_FULL, N_FULL, C), dtype=np.float32)
    out_x = np.empty_like(out_rgb)
    for core, res in enumerate(results):
        b, s = divmod(core, 2)
        # core owns stream s -> produced the OTHER stream's output, transposed
        dst = out_rgb if s == 0 else out_x
        dst[b] = res["o"].astype(np.float32).T
    return out_rgb, out_x


def kernel(rgb, x, Wkv_rgb, Wkv_x, num_heads):
    rgb = np.asarray(rgb, dtype=np.float32)
    x = np.asarray(x, dtype=np.float32)
    Wkv_rgb = np.asarray(Wkv_rgb, dtype=np.float32)
    Wkv_x = np.asarray(Wkv_x, dtype=np.float32)
    assert int(num_heads) == H
    assert rgb.shape == (B_FULL, N_FULL, C) and x.shape == (B_FULL, N_FULL, C)

    nc = _get_module()
    in_maps = make_in_maps(rgb, x, Wkv_rgb, Wkv_x)
    res = run_bass_kernel_spmd(nc, in_maps, core_ids=list(range(8)))
    return assemble(res.results)
